# revision 1
# baseline (speedup 1.0000x reference)
"""Trainium2 Bass kernel for a 2-layer GATv2 aggregator (N=50000, E=800000).

Self-contained: kernel(**inputs) takes full inputs, shards across 8
NeuronCores internally, returns the full (50000, 128) float32 output.
"""
"""GATv2 2-layer Trainium kernel: host prep + bass program builder + runner.

Strategy (8-core SPMD):
- dst-shard nodes across cores; edges sorted by dst, grouped into 128-edge
  chunks per (128-node block, src-table-half).
- xl[src] per-edge rows fetched with dma_gather (int16 idx -> table halves).
- xr[dst] broadcast per edge via one-hot matmul from SBUF-resident xr shard.
- segment softmax denominator + message aggregation via one-hot scatter
  matmul accumulated in PSUM per node block (max-subtraction skipped:
  mathematically identical softmax, alphas are O(5)).
- AllGather (DRAM collective) shares per-shard xl tables between layers.
"""
import numpy as np
import ml_dtypes

import concourse.bass as bass
import concourse.bacc as bacc
import concourse.mybir as mybir
from concourse.tile import TileContext

BF16 = ml_dtypes.bfloat16
F32 = mybir.dt.float32
BF = mybir.dt.bfloat16
I16 = mybir.dt.int16
PAD_DST = 200.0
P = 128
CPC = 8          # chunks per gather call
NI = CPC * 128   # indices per gather call


class Cfg:
    def __init__(self, N, E, nblk, feat=128, heads1=2):
        self.N, self.E = N, E
        self.NBLK = nblk
        self.SHARD = nblk * P
        self.NPAD = 8 * self.SHARD
        assert self.NPAD >= N and self.NPAD % 256 == 0
        self.HALF = self.NPAD // 2
        assert self.HALF <= 32767
        self.F = feat
        self.H1 = heads1
        self.C1 = feat // heads1


def host_prep(cfg, x, edge_index):
    """Returns (per_core_inputs: list of dict, struct: dict)."""
    N, E = cfg.N, cfg.E
    src = np.concatenate([np.asarray(edge_index[0]), np.arange(N)]).astype(np.int64)
    dst = np.concatenate([np.asarray(edge_index[1]), np.arange(N)]).astype(np.int64)
    ET = src.shape[0]

    core = dst // cfg.SHARD
    block = (dst % cfg.SHARD) // P
    dloc = dst % P
    half = (src >= cfg.HALF).astype(np.int64)
    gval = (src - half * cfg.HALF).astype(np.int64)

    # group = (core, half, block); rank within group
    key = (core * 2 + half) * cfg.NBLK + block
    order = np.argsort(key, kind="stable")
    key_s = key[order]
    # counts per group
    ngroups = 8 * 2 * cfg.NBLK
    cnt = np.bincount(key_s, minlength=ngroups)
    starts = np.zeros(ngroups + 1, np.int64)
    np.cumsum(cnt, out=starts[1:])
    rank = np.arange(ET) - starts[key_s]

    cnt3 = cnt.reshape(8, 2, cfg.NBLK)
    S_A = int(np.ceil(cnt3[:, 0, :].max() / P))
    S_B = int(np.ceil(cnt3[:, 1, :].max() / P))
    S_A, S_B = max(S_A, 1), max(S_B, 1)
    CHA = -(-(cfg.NBLK * S_A) // CPC) * CPC
    CHB = -(-(cfg.NBLK * S_B) // CPC) * CPC
    CH = CHA + CHB
    CALLS = CH // CPC

    # chunk -> (half, block) static structure
    chunk_half = np.zeros(CH, np.int64)
    chunk_block = np.zeros(CH, np.int64)
    for c in range(CH):
        if c < CHA:
            chunk_half[c] = 0
            chunk_block[c] = min(c // S_A, cfg.NBLK - 1)
        else:
            chunk_half[c] = 1
            chunk_block[c] = min((c - CHA) // S_B, cfg.NBLK - 1)

    # fill per-core edge arrays
    gidx = np.zeros((8, CH, P), np.int16)
    dstl = np.full((8, CH, P), PAD_DST, np.float32)
    g_half = half[order]
    g_core = core[order]
    g_block = block[order]
    slot_base = np.where(g_half == 0, g_block * S_A, CHA + g_block * S_B)
    slot = slot_base + rank // P
    pos = rank % P
    gidx[g_core, slot, pos] = gval[order].astype(np.int16)
    dstl[g_core, slot, pos] = dloc[order].astype(np.float32)

    # wrap gather indices per call: local idx i -> [i%16, i//16]
    # gidx [8, CALLS, NI] -> per call [NI//16,16].T -> [16, NI//16]
    gw = gidx.reshape(8, CALLS, NI // 16, 16).transpose(0, 1, 3, 2)  # [8,CALLS,16,64]
    gw = gw.transpose(0, 2, 1, 3).reshape(8, 16, CALLS * (NI // 16))
    gw = np.tile(gw, (1, 8, 1))  # replicate to 128 partitions

    struct = dict(S_A=S_A, S_B=S_B, CHA=CHA, CHB=CHB, CH=CH, CALLS=CALLS,
                  chunk_half=chunk_half, chunk_block=chunk_block)

    x_pad = np.zeros((cfg.NPAD, cfg.F), np.float32)
    x_pad[:N] = np.asarray(x, np.float32)

    per_core = []
    for k in range(8):
        per_core.append(dict(
            xTs=np.ascontiguousarray(x_pad[k * cfg.SHARD:(k + 1) * cfg.SHARD].T),
            gidx=np.ascontiguousarray(gw[k]),
            dstl=np.ascontiguousarray(dstl[k].T.astype(BF16)),  # [128, CH]
        ))
    return per_core, struct


def host_consts(cfg, Wl1, Wr1, att1, b1, Wl2, Wr2, att2, b2):
    f = cfg.F
    c = {}
    c["w1"] = np.hstack([np.asarray(Wl1, np.float32), np.asarray(Wr1, np.float32)])
    c["w2"] = np.hstack([np.asarray(Wl2, np.float32), np.asarray(Wr2, np.float32)])
    c["attb1"] = np.tile(np.asarray(att1, np.float32).reshape(1, f), (P, 1)).astype(BF16)
    c["attb2"] = np.tile(np.asarray(att2, np.float32).reshape(1, f), (P, 1)).astype(BF16)
    c["bb1"] = np.tile(np.asarray(b1, np.float32).reshape(1, f), (P, 1))
    c["bb2"] = np.tile(np.asarray(b2, np.float32).reshape(1, f), (P, 1))
    c["iotaF"] = np.tile(np.arange(P, dtype=np.float32).reshape(1, P), (P, 1)).astype(BF16)
    c["iotaP"] = np.tile(np.arange(P, dtype=np.float32).reshape(P, 1), (1, P))
    c["iotaPB"] = c["iotaP"].astype(BF16)
    c["identB"] = np.eye(P, dtype=np.float32).astype(BF16)
    c["identF"] = np.eye(P, dtype=np.float32)
    return c


def _ap(base, layout):
    return bass.AP(base.tensor, base.offset, [list(d) for d in layout])


def build_program(cfg, struct):
    NBLK, SHARD, NPAD, HALF, F = cfg.NBLK, cfg.SHARD, cfg.NPAD, cfg.HALF, cfg.F
    CH, CALLS = struct["CH"], struct["CALLS"]
    chunk_half, chunk_block = struct["chunk_half"], struct["chunk_block"]

    nc = bacc.Bacc("TRN2", target_bir_lowering=False, debug=False,
                   num_devices=8, num_swdge_queues=2)

    # I/O
    xTs = nc.dram_tensor("xTs", [P, SHARD], F32, kind="ExternalInput")
    gidx = nc.dram_tensor("gidx", [P, CALLS * (NI // 16)], I16, kind="ExternalInput")
    dstl = nc.dram_tensor("dstl", [P, CH], BF, kind="ExternalInput")
    w1 = nc.dram_tensor("w1", [P, 2 * F], F32, kind="ExternalInput")
    w2 = nc.dram_tensor("w2", [P, 2 * F], F32, kind="ExternalInput")
    attb1 = nc.dram_tensor("attb1", [P, F], BF, kind="ExternalInput")
    attb2 = nc.dram_tensor("attb2", [P, F], BF, kind="ExternalInput")
    bb1 = nc.dram_tensor("bb1", [P, F], F32, kind="ExternalInput")
    bb2 = nc.dram_tensor("bb2", [P, F], F32, kind="ExternalInput")
    iotaF = nc.dram_tensor("iotaF", [P, P], BF, kind="ExternalInput")
    iotaP = nc.dram_tensor("iotaP", [P, P], F32, kind="ExternalInput")
    iotaPB = nc.dram_tensor("iotaPB", [P, P], BF, kind="ExternalInput")
    identB = nc.dram_tensor("identB", [P, P], BF, kind="ExternalInput")
    identF = nc.dram_tensor("identF", [P, P], F32, kind="ExternalInput")
    out = nc.dram_tensor("out", [SHARD, F], F32, kind="ExternalOutput")

    eq = mybir.AluOpType.is_equal
    mul = mybir.AluOpType.mult
    AF = mybir.ActivationFunctionType
    AX = mybir.AxisListType.X

    with TileContext(nc) as tc:
        with (
            tc.tile_pool(name="const", bufs=1) as cpool,
            tc.tile_pool(name="big", bufs=1) as bigp,
            tc.tile_pool(name="work", bufs=1) as wp,
            tc.tile_pool(name="psum", bufs=1, space="PSUM") as pp,
            tc.tile_pool(name="dram", bufs=1, space="DRAM") as dp,
        ):
            # ---- consts ----
            def load_const(t, shape, dt):
                s = cpool.tile(shape, dt, name=t.name + "_sb")
                nc.sync.dma_start(out=s[:], in_=t[:])
                return s
            w1_sb = load_const(w1, [P, 2 * F], F32)
            w2_sb = load_const(w2, [P, 2 * F], F32)
            attb1_sb = load_const(attb1, [P, F], BF)
            attb2_sb = load_const(attb2, [P, F], BF)
            bb1_sb = load_const(bb1, [P, F], F32)
            bb2_sb = load_const(bb2, [P, F], F32)
            iotaF_sb = load_const(iotaF, [P, P], BF)
            iotaP_sb = load_const(iotaP, [P, P], F32)
            iotaPB_sb = load_const(iotaPB, [P, P], BF)
            identB_sb = load_const(identB, [P, P], BF)
            identF_sb = load_const(identF, [P, P], F32)
            xTs_sb = bigp.tile([P, SHARD], F32, name="xTs_sb", tag="bigshare")
            nc.sync.dma_start(out=xTs_sb[:], in_=xTs[:])
            gidx_sb = bigp.tile([P, CALLS * (NI // 16)], I16, name="gidx_sb")
            nc.sync.dma_start(out=gidx_sb[:], in_=gidx[:])
            dstl_sb = bigp.tile([P, CH], BF, name="dstl_sb")
            nc.sync.dma_start(out=dstl_sb[:], in_=dstl[:])

            # ---- persistent big tiles ----
            lneps_sb = cpool.tile([P, 1], F32, name="lneps_sb")
            nc.vector.memset(lneps_sb[:], float(np.log(1e-16)))
            xr1_sb = bigp.tile([P, SHARD], BF, name="xr1_sb")
            xr2_sb = bigp.tile([P, SHARD], BF, name="xr2_sb")
            hT_sb = bigp.tile([P, SHARD], F32, name="hT_sb", tag="bigshare")
            hacc = bigp.tile([P, NBLK * (F + 4)], F32, name="hacc")
            stage = bigp.tile([P, SHARD], F32, name="stage")  # xl staging / h / out

            # DRAM exchange buffers
            xl1sh = dp.tile([SHARD, F], F32, name="xl1sh")
            xl1full = dp.tile([NPAD, F], F32, name="xl1full", addr_space="Shared")
            xl2sh = dp.tile([SHARD, F], F32, name="xl2sh")
            xl2full = dp.tile([NPAD, F], F32, name="xl2full", addr_space="Shared")

            def shard_matmul(lhs_sb, w_sb, xr_dst, lhs_dt_note):
                """49x: [128n,256] = lhs_blockT.T @ [Wl|Wr]; xl->stage, xr->xr_dst(bf16)."""
                for j in range(NBLK):
                    mm = pp.tile([P, 512], F32, tag="txr", bufs=2, name=f"mm{j}")
                    nc.tensor.matmul(out=mm[:, 0:2 * F],
                                     lhsT=lhs_sb[:, j * P:(j + 1) * P],
                                     rhs=w_sb[:], start=True, stop=True)
                    nc.vector.tensor_copy(out=stage[:, j * F:(j + 1) * F], in_=mm[:, 0:F])
                    nc.vector.tensor_copy(out=xr_dst[:, j * P:(j + 1) * P], in_=mm[:, F:2 * F])

            def dma_stage_to(dram_tile):
                o = dram_tile[:].rearrange("(b p) f -> p b f", p=P)
                i = stage[:].rearrange("p (b f) -> p b f", f=F)
                nc.sync.dma_start(out=o, in_=i)

            # ---- phase 0: xl1/xr1 ----
            shard_matmul(xTs_sb, w1_sb, xr1_sb, "f32")
            dma_stage_to(xl1sh)
            nc.gpsimd.collective_compute(
                "AllGather", mybir.AluOpType.bypass,
                replica_groups=[list(range(8))],
                ins=[xl1sh[:]], outs=[xl1full[:]])

            def edge_pass(layer, table, xr_sb, attb_sb):
                HN = cfg.H1 if layer == 1 else 1
                CW = F // HN
                RW = F + 2 * HN
                nc.vector.memset(hacc[:, 0:NBLK * RW], 0.0)
                bp = None
                for g in range(CALLS):
                    cb = g * CPC
                    hf = int(chunk_half[cb])
                    tab = table[:][0:HALF, :] if hf == 0 else table[:][HALF:NPAD, :]
                    xg = wp.tile([P, CPC, F], F32, tag="xg", bufs=3, name=f"xg{layer}_{g}")
                    nc.gpsimd.dma_gather(
                        out_ap=xg[:], in_ap=tab,
                        idxs_ap=gidx_sb[:, g * (NI // 16):(g + 1) * (NI // 16)],
                        num_idxs=NI, num_idxs_reg=NI, elem_size=F,
                        queue_num=g % 2)
                    # QT[e, n] one-hot (batched over call)
                    qt = wp.tile([P, CPC, P], BF, tag="qt", bufs=3, name=f"qt{layer}_{g}")
                    d8 = dstl_sb[:, cb:cb + CPC]
                    nc.vector.tensor_tensor(
                        out=qt[:], in0=d8.to_broadcast([P, CPC, P]),
                        in1=_ap(iotaF_sb[:], [iotaF_sb[:].ap[0], [0, CPC], [1, P]]),
                        op=eq)
                    qtf = wp.tile([P, CPC, P], F32, tag="qtf", bufs=2,
                                  name=f"qtf{layer}_{g}")
                    nc.vector.tensor_tensor(
                        out=qtf[:], in0=d8.to_broadcast([P, CPC, P]),
                        in1=_ap(iotaF_sb[:], [iotaF_sb[:].ap[0], [0, CPC], [1, P]]),
                        op=eq)
                    # dstT via PE transpose; Q[n, e] one-hot
                    trp = [pp.tile([P, 512], BF, tag="trp", bufs=2, name=f"trp{layer}_{g}_{i}")
                           for i in range(2)]
                    for c in range(CPC):
                        col = dstl_sb[:, cb + c:cb + c + 1]
                        nc.tensor.transpose(
                            out=trp[c // 4][:, (c % 4) * P:(c % 4 + 1) * P],
                            in_=col.to_broadcast([P, P]), identity=identB_sb[:])
                    q = wp.tile([P, CPC, P], BF, tag="q", bufs=3, name=f"q{layer}_{g}")
                    for i in range(2):
                        nc.vector.tensor_tensor(
                            out=q[:, i * 4:(i + 1) * 4, :],
                            in0=_ap(iotaPB_sb[:], [iotaPB_sb[:].ap[0], [0, 4], [1, P]]),
                            in1=trp[i][:].rearrange("p (c f) -> p c f", f=P),
                            op=eq)
                    # xr gather matmuls
                    txr = [pp.tile([P, 512], F32, tag="txr", bufs=2, name=f"txr{layer}_{g}_{i}")
                           for i in range(2)]
                    for c in range(CPC):
                        blk = int(chunk_block[cb + c])
                        nc.tensor.matmul(
                            out=txr[c // 4][:, (c % 4) * P:(c % 4 + 1) * P],
                            lhsT=q[:, c, :], rhs=xr_sb[:, blk * P:(blk + 1) * P],
                            start=True, stop=True)
                    # t = xg + xr
                    tt = wp.tile([P, NI], F32, tag="tt", bufs=2, name=f"tt{layer}_{g}")
                    for i in range(2):
                        nc.vector.tensor_add(
                            out=tt[:, i * 512:(i + 1) * 512],
                            in0=xg[:, i * 4:(i + 1) * 4, :].rearrange("p c f -> p (c f)"),
                            in1=txr[i][:])
                    # leaky relu = relu(x) - relu(-0.2 x)
                    lra = wp.tile([P, NI], BF, tag="lra", bufs=2, name=f"lra{layer}_{g}")
                    lrb = wp.tile([P, NI], BF, tag="lrb", bufs=2, name=f"lrb{layer}_{g}")
                    nc.scalar.activation(out=lra[:], in_=tt[:], func=AF.Relu)
                    nc.scalar.activation(out=lrb[:], in_=tt[:], func=AF.Relu, scale=-0.2)
                    lr = lra
                    nc.vector.tensor_tensor(out=lr[:], in0=lra[:], in1=lrb[:],
                                            op=mybir.AluOpType.subtract)
                    # u = lr * att
                    u = wp.tile([P, NI], F32, tag="u", bufs=2, name=f"u{layer}_{g}")
                    nc.vector.tensor_tensor(
                        out=u[:].rearrange("p (c f) -> p c f", f=F),
                        in0=lr[:].rearrange("p (c f) -> p c f", f=F),
                        in1=_ap(attb_sb[:], [attb_sb[:].ap[0], [0, CPC], [1, F]]),
                        op=mul)
                    # alpha, p
                    al = wp.tile([P, CPC * HN], F32, tag="al", bufs=2, name=f"al{layer}_{g}")
                    nc.vector.reduce_sum(
                        out=al[:], in_=u[:].rearrange("p (g s) -> p g s", s=CW), axis=AX)
                    pe = wp.tile([P, CPC * HN], F32, tag="pe", bufs=2, name=f"pe{layer}_{g}")
                    nc.scalar.activation(out=pe[:], in_=al[:], func=AF.Exp)
                    # rhs = [p * xg | p]
                    RWB = F + HN
                    rhs = wp.tile([P, CPC * RWB], BF, tag="rhs", bufs=3, name=f"rhs{layer}_{g}")
                    rbase = rhs[:]
                    pbase = pe[:]
                    xbase = xg[:]
                    nc.vector.tensor_tensor(
                        out=_ap(rbase, [rbase.ap[0], [RWB, CPC], [CW, HN], [1, CW]]),
                        in0=_ap(xbase, [xbase.ap[0], [F, CPC], [CW, HN], [1, CW]]),
                        in1=_ap(pbase, [pbase.ap[0], [HN, CPC], [1, HN], [0, CW]]),
                        op=mul)
                    pc_out = bass.AP(rbase.tensor, rbase.offset + F,
                                     [list(rbase.ap[0]), [RWB, CPC], [1, HN]])
                    nc.vector.tensor_copy(
                        out=pc_out, in_=pbase.rearrange("p (c h) -> p c h", h=HN))
                    # scatter matmuls per chunk, PSUM-accumulated per block segment
                    for c in range(CPC):
                        ci = cb + c
                        blk = int(chunk_block[ci])
                        seg_start = ci == 0 or chunk_block[ci - 1] != blk
                        seg_end = ci == CH - 1 or chunk_block[ci + 1] != blk
                        if seg_start:
                            bp = pp.tile([P, RWB], F32, tag="bp", bufs=2, name=f"bp{layer}_{ci}")
                            ba = pp.tile([P, 2 * HN], F32, tag="ba", bufs=2, name=f"ba{layer}_{ci}")
                        nc.tensor.matmul(
                            out=bp[:], lhsT=qt[:, c, :],
                            rhs=rhs[:, c * RWB:(c + 1) * RWB],
                            start=seg_start, stop=seg_end)
                        nc.tensor.matmul(
                            out=ba[:, 0:HN], lhsT=qtf[:, c, :],
                            rhs=al[:, c * HN:(c + 1) * HN],
                            start=seg_start, stop=seg_end)
                        if seg_end:
                            nc.vector.tensor_add(
                                out=hacc[:, blk * RW:blk * RW + RWB],
                                in0=hacc[:, blk * RW:blk * RW + RWB], in1=bp[:])
                            nc.vector.tensor_add(
                                out=hacc[:, blk * RW + RWB:blk * RW + RW],
                                in0=hacc[:, blk * RW + RWB:blk * RW + RW], in1=ba[:, 0:HN])

            def elu_inplace(sl, tmp1, tmp2):
                nc.vector.tensor_scalar_min(out=tmp1[:], in0=sl, scalar1=0.0)
                nc.scalar.activation(out=tmp2[:], in_=tmp1[:], func=AF.Exp)
                nc.vector.tensor_scalar_max(out=sl, in0=sl, scalar1=0.0)
                nc.vector.tensor_add(out=sl, in0=sl, in1=tmp2[:])
                nc.vector.tensor_scalar_add(out=sl, in0=sl, scalar1=-1.0)

            def epilogue(layer, bb_sb):
                HN = cfg.H1 if layer == 1 else 1
                CW = F // HN
                RW = F + 2 * HN
                LN_EPS = float(np.log(1e-16))
                for b in range(NBLK):
                    eps = wp.tile([P, HN], F32, tag="eps", bufs=2, name=f"eps{layer}_{b}")
                    nc.scalar.activation(
                        out=eps[:], in_=hacc[:, b * RW + F + HN:b * RW + RW],
                        func=AF.Exp, bias=lneps_sb[:, 0:1])
                    den = wp.tile([P, HN], F32, tag="den", bufs=2, name=f"den{layer}_{b}")
                    nc.vector.tensor_add(
                        out=den[:], in0=hacc[:, b * RW + F:b * RW + F + HN], in1=eps[:])
                    rec = wp.tile([P, HN], F32, tag="rec", bufs=2, name=f"rec{layer}_{b}")
                    nc.vector.reciprocal(out=rec[:], in_=den[:])
                    sl = stage[:, b * F:(b + 1) * F]
                    for h in range(HN):
                        nc.vector.tensor_tensor(
                            out=stage[:, b * F + h * CW:b * F + (h + 1) * CW],
                            in0=hacc[:, b * RW + h * CW:b * RW + (h + 1) * CW],
                            in1=rec[:, h:h + 1].to_broadcast([P, CW]), op=mul)
                    nc.vector.tensor_add(out=sl, in0=sl, in1=bb_sb[:])
                    tmp1 = wp.tile([P, F], F32, tag="tmp1", bufs=2, name=f"t1_{layer}_{b}")
                    tmp2 = wp.tile([P, F], F32, tag="tmp2", bufs=2, name=f"t2_{layer}_{b}")
                    elu_inplace(sl, tmp1, tmp2)
                    if layer == 1:
                        trh = pp.tile([P, 512], F32, tag="trp", bufs=2, name=f"trh{b}")
                        nc.tensor.transpose(out=trh[:, 0:P], in_=sl, identity=identF_sb[:])
                        nc.vector.tensor_copy(out=hT_sb[:, b * P:(b + 1) * P], in_=trh[:, 0:P])

            # ---- layer 1 ----
            edge_pass(1, xl1full, xr1_sb, attb1_sb)
            epilogue(1, bb1_sb)
            # ---- phase 2: xl2/xr2 from hT ----
            shard_matmul(hT_sb, w2_sb, xr2_sb, "f32")
            dma_stage_to(xl2sh)
            nc.gpsimd.collective_compute(
                "AllGather", mybir.AluOpType.bypass,
                replica_groups=[list(range(8))],
                ins=[xl2sh[:]], outs=[xl2full[:]])
            # ---- layer 2 ----
            edge_pass(2, xl2full, xr2_sb, attb2_sb)
            epilogue(2, bb2_sb)
            # write out
            oo = out[:].rearrange("(b p) f -> p b f", p=P)
            ii = stage[:].rearrange("p (b f) -> p b f", f=F)
            nc.sync.dma_start(out=oo, in_=ii)

    nc.compile()
    return nc


def run(cfg, inputs, trace=False, core_results=False):
    from concourse.bass_utils import run_bass_kernel_spmd
    x = np.asarray(inputs["x"], np.float32)
    ei = np.asarray(inputs["edge_index"])
    per_core, struct = host_prep(cfg, x, ei)
    consts = host_consts(cfg, *[inputs[k] for k in
                                ("Wl1", "Wr1", "att1", "b1", "Wl2", "Wr2", "att2", "b2")])
    nc = build_program(cfg, struct)
    in_maps = []
    for k in range(8):
        m = dict(per_core[k])
        m.update(consts)
        in_maps.append(m)
    res = run_bass_kernel_spmd(nc, in_maps, core_ids=list(range(8)), trace=trace)
    outs = [res.results[k]["out"] for k in range(8)]
    full = np.concatenate(outs, axis=0)[:cfg.N]
    return full, res


# ---------------------------------------------------------------------------
# public entry point
# ---------------------------------------------------------------------------
_CACHE = {}
LAST_RESULTS = None


def _trace_enabled():
    import os
    return os.environ.get("GAT_TRACE", "") == "1"


def _install_trace_shim():
    """antenv.axon_hooks is absent in this image; recreate it so trace=True
    can capture NTFF profiles through the axon PJRT plugin."""
    import sys, types
    if "antenv.axon_hooks" in sys.modules:
        return
    try:
        mod = types.ModuleType("antenv.axon_hooks")
        mod._hook = None
        mod.set_axon_ntff_profile_hook = lambda h: setattr(mod, "_hook", h)
        mod.get_axon_ntff_profile_hook = lambda: mod._hook
        sys.modules["antenv.axon_hooks"] = mod
        import antenv
        antenv.axon_hooks = mod
        from trn_agent_boot.trn_boot import _ntff_profile_via_ctypes
        mod._hook = _ntff_profile_via_ctypes("/opt/axon/libaxon_pjrt.so")
        import concourse.bass_utils as bu
        bu.upload_artifacts = lambda tmpdir: str(tmpdir)
    except Exception:
        pass


def kernel(x, edge_index, Wl1, Wr1, att1, b1, Wl2, Wr2, att2, b2):
    global LAST_RESULTS
    from concourse.bass_utils import run_bass_kernel_spmd

    trace = _trace_enabled()
    if trace:
        _install_trace_shim()

    x = np.asarray(x, np.float32)
    edge_index = np.asarray(edge_index)
    N, E = x.shape[0], edge_index.shape[1]
    cfg = Cfg(N, E, nblk=49)

    per_core, struct = host_prep(cfg, x, edge_index)
    consts = host_consts(cfg, Wl1, Wr1, att1, b1, Wl2, Wr2, att2, b2)

    key = (N, E, x.shape[1], struct["S_A"], struct["S_B"])
    if key not in _CACHE:
        _CACHE[key] = build_program(cfg, struct)
    nc = _CACHE[key]

    in_maps = []
    for k in range(8):
        m = dict(per_core[k])
        m.update(consts)
        in_maps.append(m)
    res = run_bass_kernel_spmd(nc, in_maps, core_ids=list(range(8)), trace=trace)
    LAST_RESULTS = res
    outs = [np.asarray(res.results[k]["out"]) for k in range(8)]
    return np.concatenate(outs, axis=0)[:N].astype(np.float32)

